# revision 28
# baseline (speedup 1.0000x reference)
"""Trainium2 Bass kernel for transfer-matrix reflectometry (DifferentiableKLA).

Math: for each batch row b (7 film thicknesses) and wavelength w, the
reference computes R = |M10/M00|^2 of an 8-interface transfer-matrix
product. Dividing the backward (Rouard) recurrence through by v and
rescaling by conj(E_i) each step (the overall complex scale cancels in the
ratio) gives the division-free form over x = 1/lambda:

    u, v   <- r_8, 1
    for i = 7..1:   T = v * conj(E_i);  u' = r_i*T + u;  v' = T + r_i*u
    R = |u|^2 / |v|^2,   E_i = exp(i * theta_i), theta_i = 2*pi*x*g_i,
    g_i = 2*n_i*d_i,   r_i = (n_{i-1}-n_i)/(n_{i-1}+n_i)  (real scalars
    since the refractive indices are wavelength-constant and real).

Key accel trick: U2=|u|^2 and V2=|v|^2 are *exactly band-limited* in x
(finite exponential sums, |freq| <= 4*pi*sum_j g_j), so they are computed
on a coarse WC-point grid in x and upsampled to the 801-point grid with a
precomputed band-limited least-squares matrix — evaluated as a TensorE
matmul that simultaneously transposes to the [batch, wavelength] output
layout. Only the final reciprocal+multiply runs on the fine grid.

Layout: coarse grid on partitions, batch along the free dim, two
independent batch halves packed as 2 x WC=64 partition groups (rank-2
matmul builds theta for both halves at once).
"""

import math

import numpy as np

W = 801
B = 16384
NCORES = 8
BPC = B // NCORES  # 2048 batch rows per core
WC = 64  # coarse wavelength nodes
PACK = 2  # batch halves packed along partitions
NPART = WC * PACK  # 128
NB = 1024  # free-dim columns per compute chunk
NCHUNK = BPC // PACK // NB  # 1
EXT = 0.05  # coarse-grid extension fraction beyond the x band
REG = 1e-10  # LS ridge regularization
MFREQ = 6000  # frequency samples for the LS fit
TWO_PI = 2.0 * math.pi
# worst-case band limit for d < 200nm stacks (4*pi*sum(2*n_j*200)), padded
B_DESIGN = 2.0 * TWO_PI * (2.0 * (1.46 * 4 + 2.0 * 3) * 200.0) * 1.02

_interp_cache: dict = {}
_program_cache: dict = {}
_last_launch: dict = {}  # debug/profiling hook (nc, in_maps of last run)


def _build_interp(x_fine: np.ndarray, bfit: float):
    """Band-limited LS upsampling matrix A [W, WC]: rows map coarse samples
    (at xc) of any signal with spectrum in [-bfit, bfit] to the fine grid."""
    key = (x_fine.shape[0], float(x_fine[0]), float(x_fine[-1]), round(bfit))
    if key in _interp_cache:
        return _interp_cache[key]
    x0, x1 = float(x_fine.min()), float(x_fine.max())
    ln = x1 - x0
    xc = np.linspace(x0 - EXT * ln, x1 + EXT * ln, WC)
    f = np.linspace(0.0, bfit, MFREQ)
    s_mat = np.concatenate(
        [np.cos(np.outer(xc - x0, f)), np.sin(np.outer(xc - x0, f))], axis=1
    )
    t_mat = np.concatenate(
        [np.cos(np.outer(x_fine - x0, f)), np.sin(np.outer(x_fine - x0, f))],
        axis=1,
    )
    g = s_mat @ s_mat.T
    a = t_mat @ s_mat.T @ np.linalg.inv(
        g + REG * np.trace(g) / WC * np.eye(WC)
    )
    res = (xc, np.ascontiguousarray(a, dtype=np.float32))
    _interp_cache[key] = res
    return res


def _build_program(r: tuple):
    """Build the per-core Bass program (SPMD; identical on all cores).
    `r` are the 8 Fresnel coefficients, baked in as immediates."""
    if r in _program_cache:
        return _program_cache[r]

    import concourse.tile as tile
    from concourse import bacc, mybir

    f32 = mybir.dt.float32
    alu = mybir.AluOpType
    act_sin = mybir.ActivationFunctionType.Sin

    # Bacc (not raw Bass): its compile() splits multi-sem waits into event
    # semaphores (TRN2 allows 1 wait/instruction), moves matmul waits to
    # ldweights, inserts ACT table loads, and fills extended-ISA bytes.
    nc = bacc.Bacc()
    # kg[h, :]: cols 0..NPART-1 = k0 pattern for half h; then per layer i,
    # cols NPART+i*(BPC//PACK) .. +BPC//PACK = that layer's g for half h.
    # One param -> one DMA -> one wait on the PE absorber below.
    kgw = NPART + 7 * (BPC // PACK)
    k_g = nc.declare_dram_parameter("kg", [PACK, kgw], f32, isOutput=False)
    # A^T duplicated into both partition halves (rows 0:WC and WC:2*WC)
    amat = nc.declare_dram_parameter("Amat2", [NPART, W], f32, isOutput=False)
    out = nc.declare_dram_parameter("out", [BPC, W], f32, isOutput=True)

    with tile.TileContext(nc) as tc:
        with (
            tc.tile_pool(name="const", bufs=1) as cpool,
            tc.tile_pool(name="state", bufs=2) as spool,
            tc.tile_pool(name="work", bufs=2) as wpool,
            tc.tile_pool(name="thps", bufs=2, space="PSUM") as ppool,
            tc.tile_pool(name="upps", bufs=2, space="PSUM") as upool,
            tc.tile_pool(name="outp", bufs=3) as opool,
        ):
            kg_sb = cpool.tile([PACK, kgw], f32)
            nc.sync.dma_start(out=kg_sb, in_=k_g[:, :])
            a_sb = cpool.tile([NPART, W], f32)
            nc.sync.dma_start(out=a_sb, in_=amat[:, :])
            # PE supports only one sync wait per matmul: absorb each const
            # DMA with a dummy matmul so real matmuls never wait on DMA sems
            dummy = upool.tile([PACK, 8], f32, tag="upu")
            nc.tensor.matmul(dummy, kg_sb[:, :PACK], kg_sb[:, :8], start=True, stop=True)
            dummy2 = upool.tile([NPART, 8], f32, tag="upv")
            nc.tensor.matmul(dummy2, a_sb[:, :NPART], a_sb[:, :8], start=True, stop=True)
            # scheduler-only fence (no semaphores): keep the absorbers first
            tc.no_sync_barrier()

            for chunk in range(NCHUNK):
                ur = spool.tile([NPART, NB], f32, tag="ur")
                ui = spool.tile([NPART, NB], f32, tag="ui")
                vr = spool.tile([NPART, NB], f32, tag="vr")
                vi = spool.tile([NPART, NB], f32, tag="vi")
                nc.vector.memset(ur, float(r[7]))
                nc.vector.memset(ui, 0.0)
                nc.vector.memset(vr, 1.0)
                nc.vector.memset(vi, 0.0)

                for i in range(7, 0, -1):
                    ri = float(r[i - 1])
                    # theta[p, n] = k0c[p mod WC] * g_half(p)[n] (rank-2 mm;
                    # split 512-wide: a matmul may not cross a PSUM bank)
                    theta = ppool.tile([NPART, NB], f32, tag="theta")
                    g0 = NPART + (i - 1) * (BPC // PACK) + chunk * NB
                    for t0 in range(0, NB, 512):
                        nc.tensor.matmul(
                            theta[:, t0 : t0 + 512],
                            kg_sb[:, :NPART],
                            kg_sb[:, g0 + t0 : g0 + t0 + 512],
                            start=True,
                            stop=True,
                        )
                    # range-reduce into [-pi, pi]: theta in [0, ~4.2*pi)
                    psi_s = wpool.tile([NPART, NB], f32, tag="psis")
                    nc.vector.add_range_wrap(
                        psi_s, theta, shift=-TWO_PI, bound=math.pi, period=TWO_PI
                    )
                    psi_c = wpool.tile([NPART, NB], f32, tag="psic")
                    nc.vector.add_range_wrap(
                        psi_c,
                        theta,
                        shift=0.5 * math.pi - TWO_PI,
                        bound=math.pi,
                        period=TWO_PI,
                    )
                    s_t = wpool.tile([NPART, NB], f32, tag="s")
                    nc.scalar.activation(s_t, psi_s, act_sin)
                    c_t = wpool.tile([NPART, NB], f32, tag="c")
                    nc.scalar.activation(c_t, psi_c, act_sin)

                    # T = v * conj(E) : Tr = vr*c + vi*s ; Ti = vi*c - vr*s
                    p1 = wpool.tile([NPART, NB], f32, tag="p1")
                    nc.vector.tensor_mul(p1, vr, c_t)
                    p2 = wpool.tile([NPART, NB], f32, tag="p2")
                    nc.vector.tensor_mul(p2, vi, s_t)
                    # imag-T chain on the otherwise-idle GpSimd engine
                    p3 = wpool.tile([NPART, NB], f32, tag="p3")
                    nc.gpsimd.tensor_mul(p3, vi, c_t)
                    p4 = wpool.tile([NPART, NB], f32, tag="p4")
                    nc.gpsimd.tensor_mul(p4, vr, s_t)
                    t_r = wpool.tile([NPART, NB], f32, tag="Tr")
                    nc.vector.tensor_add(t_r, p1, p2)
                    t_i = wpool.tile([NPART, NB], f32, tag="Ti")
                    nc.gpsimd.tensor_sub(t_i, p3, p4)

                    # u' = r*T + u ; v' = T + r*u   (old u!)
                    ur2 = spool.tile([NPART, NB], f32, tag="ur")
                    nc.vector.scalar_tensor_tensor(
                        ur2, t_r, ri, ur, alu.mult, alu.add
                    )
                    ui2 = spool.tile([NPART, NB], f32, tag="ui")
                    nc.vector.scalar_tensor_tensor(
                        ui2, t_i, ri, ui, alu.mult, alu.add
                    )
                    vr2 = spool.tile([NPART, NB], f32, tag="vr")
                    nc.vector.scalar_tensor_tensor(
                        vr2, ur, ri, t_r, alu.mult, alu.add
                    )
                    vi2 = spool.tile([NPART, NB], f32, tag="vi")
                    nc.vector.scalar_tensor_tensor(
                        vi2, ui, ri, t_i, alu.mult, alu.add
                    )
                    ur, ui, vr, vi = ur2, ui2, vr2, vi2

                # U2 = ur^2 + ui^2 ; V2 = vr^2 + vi^2
                m1 = wpool.tile([NPART, NB], f32, tag="m1")
                nc.vector.tensor_mul(m1, ur, ur)
                m2 = wpool.tile([NPART, NB], f32, tag="m2")
                nc.vector.tensor_mul(m2, ui, ui)
                u2 = spool.tile([NPART, NB], f32, tag="u2")
                nc.vector.tensor_add(u2, m1, m2)
                m3 = wpool.tile([NPART, NB], f32, tag="m3")
                nc.vector.tensor_mul(m3, vr, vr)
                m4 = wpool.tile([NPART, NB], f32, tag="m4")
                nc.vector.tensor_mul(m4, vi, vi)
                v2 = spool.tile([NPART, NB], f32, tag="v2")
                nc.vector.tensor_add(v2, m3, m4)

                # upsample both planes per 128-row batch group; divide; store
                for half in range(PACK):
                    for bsub in range(NB // 128):
                        row0 = half * (BPC // 2) + chunk * NB + bsub * 128
                        lu = u2[half * WC : (half + 1) * WC, bsub * 128 : (bsub + 1) * 128]
                        lv = v2[half * WC : (half + 1) * WC, bsub * 128 : (bsub + 1) * 128]
                        a_half = a_sb[half * WC : (half + 1) * WC, :]
                        for w0, ws in ((0, 512), (512, W - 512)):
                            up_u = upool.tile([128, ws], f32, tag="upu")
                            nc.tensor.matmul(
                                up_u, lu, a_half[:, w0 : w0 + ws], start=True, stop=True
                            )
                            up_v = upool.tile([128, ws], f32, tag="upv")
                            nc.tensor.matmul(
                                up_v, lv, a_half[:, w0 : w0 + ws], start=True, stop=True
                            )
                            inv = opool.tile([128, ws], f32, tag="inv")
                            nc.vector.reciprocal_approx_fast(out=inv, in_=up_v)
                            ro = opool.tile([128, ws], f32, tag="ro")
                            nc.vector.tensor_mul(ro, up_u, inv)
                            nc.sync.dma_start(
                                out=out[row0 : row0 + 128, w0 : w0 + ws], in_=ro
                            )

    # the axon/bass2jax run path asserts finalized; Bacc.finalize() also runs
    # compile() (event-sem wait splitting, reg alloc, ACT table loads, ISA bytes)
    nc.finalize()

    _program_cache[r] = nc
    return nc


def _scalar_index(arr: np.ndarray, name: str) -> float:
    """Wavelength-constant real refractive index from a [W] complex array."""
    a = np.asarray(arr)
    v = complex(a.flat[0])
    if not np.allclose(a, v, rtol=1e-6, atol=1e-6):
        raise ValueError(f"{name}: kernel requires wavelength-constant index")
    if abs(v.imag) > 1e-6 * max(1.0, abs(v.real)):
        raise ValueError(f"{name}: kernel requires a real refractive index")
    return float(v.real)


def kernel(d_phys, n_Si, n_SiO2, n_Si3N4, lam):
    from concourse.bass_utils import run_bass_kernel_spmd

    d = np.ascontiguousarray(np.asarray(d_phys), dtype=np.float32)
    assert d.shape == (B, 7), d.shape
    lam_np = np.asarray(lam, dtype=np.float64)
    assert lam_np.shape == (W,)

    nsi = _scalar_index(n_Si, "n_Si")
    nox = _scalar_index(n_SiO2, "n_SiO2")
    nni = _scalar_index(n_Si3N4, "n_Si3N4")
    layers = [1.0, nox, nni, nox, nni, nox, nni, nox, nsi]
    r = tuple(
        (layers[i - 1] - layers[i]) / (layers[i - 1] + layers[i])
        for i in range(1, 9)
    )

    # g = 2*n_i*d_i per layer, [B, 7]
    nvec = np.array(layers[1:8], dtype=np.float64)
    g = (2.0 * d.astype(np.float64) * nvec[None, :]).astype(np.float32)

    x_fine = 1.0 / lam_np
    bfit_actual = 2.0 * TWO_PI * float(g.astype(np.float64).sum(axis=1).max())
    bfit = max(B_DESIGN, bfit_actual * 1.02)
    xc, a_fine = _build_interp(x_fine, bfit)  # a_fine: [W, WC]

    # sanity: wrapped phase args must stay within +-3*pi for the one-period
    # range wrap (theta in [0, theta_max])
    theta_max = TWO_PI * float(xc.max()) * float(g.max())
    assert theta_max <= 5.0 * math.pi - 0.2, theta_max

    k0c = (TWO_PI * xc).astype(np.float32)
    amat2 = np.concatenate([a_fine.T, a_fine.T], axis=0)  # [2*WC, W]
    amat2 = np.ascontiguousarray(amat2)

    kgw = NPART + 7 * (BPC // PACK)
    g_all = g.reshape(NCORES, PACK, BPC // PACK, 7)
    in_maps = []
    for c in range(NCORES):
        kg = np.zeros((PACK, kgw), dtype=np.float32)
        kg[0, :WC] = k0c
        kg[1, WC:NPART] = k0c
        for i in range(7):
            s = NPART + i * (BPC // PACK)
            kg[:, s : s + BPC // PACK] = g_all[c, :, :, i]
        in_maps.append({"kg": kg, "Amat2": amat2})

    nc = _build_program(r)
    _last_launch["nc"] = nc
    _last_launch["in_maps"] = in_maps
    res = run_bass_kernel_spmd(nc, in_maps, list(range(NCORES)))
    outs = [res.results[c]["out"] for c in range(NCORES)]
    return np.concatenate(outs, axis=0)


# revision 36
# speedup vs baseline: 1.2259x; 1.2259x over previous
"""Trainium2 Bass kernel for transfer-matrix reflectometry (DifferentiableKLA).

Math: for each batch row b (7 film thicknesses) and wavelength w, the
reference computes R = |M10/M00|^2 of an 8-interface transfer-matrix
product. Dividing the backward (Rouard) recurrence through by v and
rescaling by conj(E_i) each step (the overall complex scale cancels in the
ratio) gives the division-free form over x = 1/lambda:

    u, v   <- r_8, 1
    for i = 7..1:   T = v * conj(E_i);  u' = r_i*T + u;  v' = T + r_i*u
    R = |u|^2 / |v|^2,   E_i = exp(i * theta_i), theta_i = 2*pi*x*g_i,
    g_i = 2*n_i*d_i,   r_i = (n_{i-1}-n_i)/(n_{i-1}+n_i)  (real scalars
    since the refractive indices are wavelength-constant and real).

Key accel trick: U2=|u|^2 and V2=|v|^2 are *exactly band-limited* in x
(finite exponential sums, |freq| <= 4*pi*sum_j g_j), so they are computed
on a coarse WC-point grid in x and upsampled to the 801-point grid with a
precomputed band-limited least-squares matrix — evaluated as a TensorE
matmul that simultaneously transposes to the [batch, wavelength] output
layout. Only the final reciprocal+multiply runs on the fine grid.

Layout: coarse grid on partitions, batch along the free dim, two
independent batch halves packed as 2 x WC=64 partition groups (rank-2
matmul builds theta for both halves at once).
"""

import math

import numpy as np

W = 801
B = 16384
NCORES = 8
BPC = B // NCORES  # 2048 batch rows per core
WC = 64  # coarse wavelength nodes
PACK = 2  # batch halves packed along partitions
NPART = WC * PACK  # 128
NB = 512  # free-dim columns per compute chunk
NCHUNK = BPC // PACK // NB  # 2
EXT = 0.05  # coarse-grid extension fraction beyond the x band
REG = 1e-10  # LS ridge regularization
MFREQ = 6000  # frequency samples for the LS fit
TWO_PI = 2.0 * math.pi
# worst-case band limit for d < 200nm stacks (4*pi*sum(2*n_j*200)), padded
B_DESIGN = 2.0 * TWO_PI * (2.0 * (1.46 * 4 + 2.0 * 3) * 200.0) * 1.02

_interp_cache: dict = {}
_program_cache: dict = {}
_last_launch: dict = {}  # debug/profiling hook (nc, in_maps of last run)


def _build_interp(x_fine: np.ndarray, bfit: float):
    """Band-limited LS upsampling matrix A [W, WC]: rows map coarse samples
    (at xc) of any signal with spectrum in [-bfit, bfit] to the fine grid."""
    key = (x_fine.shape[0], float(x_fine[0]), float(x_fine[-1]), round(bfit))
    if key in _interp_cache:
        return _interp_cache[key]
    x0, x1 = float(x_fine.min()), float(x_fine.max())
    ln = x1 - x0
    xc = np.linspace(x0 - EXT * ln, x1 + EXT * ln, WC)
    f = np.linspace(0.0, bfit, MFREQ)
    s_mat = np.concatenate(
        [np.cos(np.outer(xc - x0, f)), np.sin(np.outer(xc - x0, f))], axis=1
    )
    t_mat = np.concatenate(
        [np.cos(np.outer(x_fine - x0, f)), np.sin(np.outer(x_fine - x0, f))],
        axis=1,
    )
    g = s_mat @ s_mat.T
    a = t_mat @ s_mat.T @ np.linalg.inv(
        g + REG * np.trace(g) / WC * np.eye(WC)
    )
    res = (xc, np.ascontiguousarray(a, dtype=np.float32))
    _interp_cache[key] = res
    return res


def _build_program(r: tuple):
    """Build the per-core Bass program (SPMD; identical on all cores).
    `r` are the 8 Fresnel coefficients, baked in as immediates."""
    if r in _program_cache:
        return _program_cache[r]

    import concourse.tile as tile
    from concourse import bacc, mybir

    f32 = mybir.dt.float32
    alu = mybir.AluOpType
    act_sin = mybir.ActivationFunctionType.Sin

    # Bacc (not raw Bass): its compile() splits multi-sem waits into event
    # semaphores (TRN2 allows 1 wait/instruction), moves matmul waits to
    # ldweights, inserts ACT table loads, and fills extended-ISA bytes.
    nc = bacc.Bacc()
    # kg[h, :]: cols 0..NPART-1 = k0 pattern for half h; then per layer i,
    # cols NPART+i*(BPC//PACK) .. +BPC//PACK = that layer's g for half h.
    # One param -> one DMA -> one wait on the PE absorber below.
    kgw = NPART + 7 * (BPC // PACK)
    k_g = nc.declare_dram_parameter("kg", [PACK, kgw], f32, isOutput=False)
    # A^T duplicated into both partition halves (rows 0:WC and WC:2*WC),
    # split hi/lo in bf16 ([0]=hi, [1]=lo) for the 3-term bf16 upsample
    bf16 = mybir.dt.bfloat16
    amat = nc.declare_dram_parameter("Amat2", [2, NPART, W], bf16, isOutput=False)
    out = nc.declare_dram_parameter("out", [BPC, W], f32, isOutput=True)

    with tile.TileContext(nc) as tc:
        with (
            tc.tile_pool(name="const", bufs=1) as cpool,
            tc.tile_pool(name="state", bufs=2) as spool,
            tc.tile_pool(name="work", bufs=2) as wpool,
            tc.tile_pool(name="thps", bufs=2, space="PSUM") as ppool,
            tc.tile_pool(name="upps", bufs=2, space="PSUM") as upool,
            tc.tile_pool(name="outp", bufs=3) as opool,
        ):
            kg_sb = cpool.tile([PACK, kgw], f32)
            nc.sync.dma_start(out=kg_sb, in_=k_g[:, :])
            a_sb = cpool.tile([NPART, 2, W], bf16)
            nc.sync.dma_start(
                out=a_sb, in_=amat.rearrange("s p w -> p s w")
            )
            # PE supports only one sync wait per matmul: absorb each const
            # DMA with a dummy matmul so real matmuls never wait on DMA sems
            dummy = upool.tile([PACK, 8], f32, tag="upu")
            nc.tensor.matmul(dummy, kg_sb[:, :PACK], kg_sb[:, :8], start=True, stop=True)
            dummy2 = upool.tile([NPART, 8], f32, tag="upv")
            nc.tensor.matmul(
                dummy2, a_sb[:, 0, :NPART], a_sb[:, 0, :8], start=True, stop=True
            )
            # scheduler-only fence (no semaphores): keep the absorbers first
            tc.no_sync_barrier()

            for chunk in range(NCHUNK):
                ur = spool.tile([NPART, NB], f32, tag="ur")
                ui = spool.tile([NPART, NB], f32, tag="ui")
                vr = spool.tile([NPART, NB], f32, tag="vr")
                vi = spool.tile([NPART, NB], f32, tag="vi")
                nc.vector.memset(ur, float(r[7]))
                nc.vector.memset(ui, 0.0)
                nc.vector.memset(vr, 1.0)
                nc.vector.memset(vi, 0.0)

                for i in range(7, 0, -1):
                    ri = float(r[i - 1])
                    # theta[p, n] = k0c[p mod WC] * g_half(p)[n] (rank-2 mm;
                    # split 512-wide: a matmul may not cross a PSUM bank)
                    theta = ppool.tile([NPART, NB], f32, tag="theta")
                    g0 = NPART + (i - 1) * (BPC // PACK) + chunk * NB
                    for t0 in range(0, NB, 512):
                        nc.tensor.matmul(
                            theta[:, t0 : t0 + 512],
                            kg_sb[:, :NPART],
                            kg_sb[:, g0 + t0 : g0 + t0 + 512],
                            start=True,
                            stop=True,
                        )
                    # range-reduce into [-pi, pi]: theta in [0, ~4.2*pi)
                    psi_s = wpool.tile([NPART, NB], f32, tag="psis")
                    nc.vector.add_range_wrap(
                        psi_s, theta, shift=-TWO_PI, bound=math.pi, period=TWO_PI
                    )
                    psi_c = wpool.tile([NPART, NB], f32, tag="psic")
                    nc.vector.add_range_wrap(
                        psi_c,
                        theta,
                        shift=0.5 * math.pi - TWO_PI,
                        bound=math.pi,
                        period=TWO_PI,
                    )
                    s_t = wpool.tile([NPART, NB], f32, tag="s")
                    nc.scalar.activation(s_t, psi_s, act_sin)
                    c_t = wpool.tile([NPART, NB], f32, tag="c")
                    nc.scalar.activation(c_t, psi_c, act_sin)

                    # T = v * conj(E) : Tr = vr*c + vi*s ; Ti = vi*c - vr*s
                    p1 = wpool.tile([NPART, NB], f32, tag="p1")
                    nc.vector.tensor_mul(p1, vr, c_t)
                    p2 = wpool.tile([NPART, NB], f32, tag="p2")
                    nc.vector.tensor_mul(p2, vi, s_t)
                    # two independent muls on the otherwise-idle GpSimd
                    p3 = wpool.tile([NPART, NB], f32, tag="p3")
                    nc.gpsimd.tensor_mul(p3, vi, c_t)
                    p4 = wpool.tile([NPART, NB], f32, tag="p4")
                    nc.gpsimd.tensor_mul(p4, vr, s_t)
                    t_r = wpool.tile([NPART, NB], f32, tag="Tr")
                    nc.vector.tensor_add(t_r, p1, p2)
                    t_i = wpool.tile([NPART, NB], f32, tag="Ti")
                    nc.vector.tensor_sub(t_i, p3, p4)

                    # u' = r*T + u ; v' = T + r*u   (old u!)
                    ur2 = spool.tile([NPART, NB], f32, tag="ur")
                    nc.vector.scalar_tensor_tensor(
                        ur2, t_r, ri, ur, alu.mult, alu.add
                    )
                    ui2 = spool.tile([NPART, NB], f32, tag="ui")
                    nc.vector.scalar_tensor_tensor(
                        ui2, t_i, ri, ui, alu.mult, alu.add
                    )
                    vr2 = spool.tile([NPART, NB], f32, tag="vr")
                    nc.vector.scalar_tensor_tensor(
                        vr2, ur, ri, t_r, alu.mult, alu.add
                    )
                    vi2 = spool.tile([NPART, NB], f32, tag="vi")
                    nc.vector.scalar_tensor_tensor(
                        vi2, ui, ri, t_i, alu.mult, alu.add
                    )
                    ur, ui, vr, vi = ur2, ui2, vr2, vi2

                # U2 = ur^2 + ui^2 ; V2 = vr^2 + vi^2
                m1 = wpool.tile([NPART, NB], f32, tag="p1")
                nc.vector.tensor_mul(m1, ur, ur)
                m2 = wpool.tile([NPART, NB], f32, tag="p2")
                nc.vector.tensor_mul(m2, ui, ui)
                u2 = spool.tile([NPART, NB], f32, tag="u2")
                nc.vector.tensor_add(u2, m1, m2)
                m3 = wpool.tile([NPART, NB], f32, tag="p3")
                nc.vector.tensor_mul(m3, vr, vr)
                m4 = wpool.tile([NPART, NB], f32, tag="p4")
                nc.vector.tensor_mul(m4, vi, vi)
                v2 = spool.tile([NPART, NB], f32, tag="v2")
                nc.vector.tensor_add(v2, m3, m4)

                # split planes into bf16 hi + lo halves: the 3-term bf16
                # matmul (hi*hi + lo*hi + hi*lo) reproduces the fp32 product
                # to ~2^-16 at ~3x less PE time than fp32's LOW_HIGH mode
                u2h = spool.tile([NPART, NB], bf16, tag="u2h")
                nc.vector.tensor_copy(u2h, u2)
                u2l = spool.tile([NPART, NB], bf16, tag="u2l")
                nc.vector.tensor_sub(u2l, u2, u2h)
                v2h = spool.tile([NPART, NB], bf16, tag="v2h")
                nc.vector.tensor_copy(v2h, v2)
                v2l = spool.tile([NPART, NB], bf16, tag="v2l")
                nc.vector.tensor_sub(v2l, v2, v2h)

                # upsample both planes per 128-row batch group; divide; store
                for half in range(PACK):
                    for bsub in range(NB // 128):
                        row0 = half * (BPC // 2) + chunk * NB + bsub * 128
                        bs = slice(bsub * 128, (bsub + 1) * 128)
                        ps = slice(half * WC, (half + 1) * WC)
                        for w0, ws in ((0, 512), (512, W - 512)):
                            a_h = a_sb[ps, 0, w0 : w0 + ws]
                            a_l = a_sb[ps, 1, w0 : w0 + ws]
                            up_u = upool.tile([128, ws], f32, tag="upu")
                            nc.tensor.matmul(
                                up_u, u2h[ps, bs], a_h, start=True, stop=False
                            )
                            nc.tensor.matmul(
                                up_u, u2h[ps, bs], a_l, start=False, stop=False
                            )
                            nc.tensor.matmul(
                                up_u, u2l[ps, bs], a_h, start=False, stop=True
                            )
                            up_v = upool.tile([128, ws], f32, tag="upv")
                            nc.tensor.matmul(
                                up_v, v2h[ps, bs], a_h, start=True, stop=False
                            )
                            nc.tensor.matmul(
                                up_v, v2h[ps, bs], a_l, start=False, stop=False
                            )
                            nc.tensor.matmul(
                                up_v, v2l[ps, bs], a_h, start=False, stop=True
                            )
                            inv = opool.tile([128, ws], f32, tag="inv")
                            nc.vector.reciprocal_approx_fast(out=inv, in_=up_v)
                            ro = opool.tile([128, ws], f32, tag="ro")
                            nc.vector.tensor_mul(ro, up_u, inv)
                            nc.sync.dma_start(
                                out=out[row0 : row0 + 128, w0 : w0 + ws], in_=ro
                            )

    # the axon/bass2jax run path asserts finalized; Bacc.finalize() also runs
    # compile() (event-sem wait splitting, reg alloc, ACT table loads, ISA bytes)
    nc.finalize()

    _program_cache[r] = nc
    return nc


def _scalar_index(arr: np.ndarray, name: str) -> float:
    """Wavelength-constant real refractive index from a [W] complex array."""
    a = np.asarray(arr)
    v = complex(a.flat[0])
    if not np.allclose(a, v, rtol=1e-6, atol=1e-6):
        raise ValueError(f"{name}: kernel requires wavelength-constant index")
    if abs(v.imag) > 1e-6 * max(1.0, abs(v.real)):
        raise ValueError(f"{name}: kernel requires a real refractive index")
    return float(v.real)


def kernel(d_phys, n_Si, n_SiO2, n_Si3N4, lam):
    from concourse.bass_utils import run_bass_kernel_spmd

    d = np.ascontiguousarray(np.asarray(d_phys), dtype=np.float32)
    assert d.shape == (B, 7), d.shape
    lam_np = np.asarray(lam, dtype=np.float64)
    assert lam_np.shape == (W,)

    nsi = _scalar_index(n_Si, "n_Si")
    nox = _scalar_index(n_SiO2, "n_SiO2")
    nni = _scalar_index(n_Si3N4, "n_Si3N4")
    layers = [1.0, nox, nni, nox, nni, nox, nni, nox, nsi]
    r = tuple(
        (layers[i - 1] - layers[i]) / (layers[i - 1] + layers[i])
        for i in range(1, 9)
    )

    # g = 2*n_i*d_i per layer, [B, 7]
    nvec = np.array(layers[1:8], dtype=np.float64)
    g = (2.0 * d.astype(np.float64) * nvec[None, :]).astype(np.float32)

    x_fine = 1.0 / lam_np
    bfit_actual = 2.0 * TWO_PI * float(g.astype(np.float64).sum(axis=1).max())
    bfit = max(B_DESIGN, bfit_actual * 1.02)
    xc, a_fine = _build_interp(x_fine, bfit)  # a_fine: [W, WC]

    # sanity: wrapped phase args must stay within +-3*pi for the one-period
    # range wrap (theta in [0, theta_max])
    theta_max = TWO_PI * float(xc.max()) * float(g.max())
    assert theta_max <= 5.0 * math.pi - 0.2, theta_max

    import ml_dtypes

    k0c = (TWO_PI * xc).astype(np.float32)
    a2 = np.concatenate([a_fine.T, a_fine.T], axis=0)  # [2*WC, W] fp32
    a_hi = a2.astype(ml_dtypes.bfloat16)
    a_lo = (a2 - a_hi.astype(np.float32)).astype(ml_dtypes.bfloat16)
    amat2 = np.ascontiguousarray(np.stack([a_hi, a_lo], axis=0))

    kgw = NPART + 7 * (BPC // PACK)
    g_all = g.reshape(NCORES, PACK, BPC // PACK, 7)
    in_maps = []
    for c in range(NCORES):
        kg = np.zeros((PACK, kgw), dtype=np.float32)
        kg[0, :WC] = k0c
        kg[1, WC:NPART] = k0c
        for i in range(7):
            s = NPART + i * (BPC // PACK)
            kg[:, s : s + BPC // PACK] = g_all[c, :, :, i]
        in_maps.append({"kg": kg, "Amat2": amat2})

    nc = _build_program(r)
    _last_launch["nc"] = nc
    _last_launch["in_maps"] = in_maps
    res = run_bass_kernel_spmd(nc, in_maps, list(range(NCORES)))
    outs = [res.results[c]["out"] for c in range(NCORES)]
    return np.concatenate(outs, axis=0)


# revision 38
# speedup vs baseline: 1.2264x; 1.0004x over previous
"""Trainium2 Bass kernel for transfer-matrix reflectometry (DifferentiableKLA).

Math: for each batch row b (7 film thicknesses) and wavelength w, the
reference computes R = |M10/M00|^2 of an 8-interface transfer-matrix
product. Dividing the backward (Rouard) recurrence through by v and
rescaling by conj(E_i) each step (the overall complex scale cancels in the
ratio) gives the division-free form over x = 1/lambda:

    u, v   <- r_8, 1
    for i = 7..1:   T = v * conj(E_i);  u' = r_i*T + u;  v' = T + r_i*u
    R = |u|^2 / |v|^2,   E_i = exp(i * theta_i), theta_i = 2*pi*x*g_i,
    g_i = 2*n_i*d_i,   r_i = (n_{i-1}-n_i)/(n_{i-1}+n_i)  (real scalars
    since the refractive indices are wavelength-constant and real).

Key accel trick: U2=|u|^2 and V2=|v|^2 are *exactly band-limited* in x
(finite exponential sums, |freq| <= 4*pi*sum_j g_j), so they are computed
on a coarse WC-point grid in x and upsampled to the 801-point grid with a
precomputed band-limited least-squares matrix — evaluated as a TensorE
matmul that simultaneously transposes to the [batch, wavelength] output
layout. Only the final reciprocal+multiply runs on the fine grid.

Layout: coarse grid on partitions, batch along the free dim, two
independent batch halves packed as 2 x WC=64 partition groups (rank-2
matmul builds theta for both halves at once).
"""

import math

import numpy as np

W = 801
B = 16384
NCORES = 8
BPC = B // NCORES  # 2048 batch rows per core
WC = 64  # coarse wavelength nodes
PACK = 2  # batch halves packed along partitions
NPART = WC * PACK  # 128
NB = 1024  # free-dim columns per compute chunk
NCHUNK = BPC // PACK // NB  # 1
EXT = 0.05  # coarse-grid extension fraction beyond the x band
REG = 1e-10  # LS ridge regularization
MFREQ = 6000  # frequency samples for the LS fit
TWO_PI = 2.0 * math.pi
# worst-case band limit for d < 200nm stacks (4*pi*sum(2*n_j*200)), padded
B_DESIGN = 2.0 * TWO_PI * (2.0 * (1.46 * 4 + 2.0 * 3) * 200.0) * 1.02

_interp_cache: dict = {}
_program_cache: dict = {}
_last_launch: dict = {}  # debug/profiling hook (nc, in_maps of last run)


def _build_interp(x_fine: np.ndarray, bfit: float):
    """Band-limited LS upsampling matrix A [W, WC]: rows map coarse samples
    (at xc) of any signal with spectrum in [-bfit, bfit] to the fine grid."""
    key = (x_fine.shape[0], float(x_fine[0]), float(x_fine[-1]), round(bfit))
    if key in _interp_cache:
        return _interp_cache[key]
    x0, x1 = float(x_fine.min()), float(x_fine.max())
    ln = x1 - x0
    xc = np.linspace(x0 - EXT * ln, x1 + EXT * ln, WC)
    f = np.linspace(0.0, bfit, MFREQ)
    s_mat = np.concatenate(
        [np.cos(np.outer(xc - x0, f)), np.sin(np.outer(xc - x0, f))], axis=1
    )
    t_mat = np.concatenate(
        [np.cos(np.outer(x_fine - x0, f)), np.sin(np.outer(x_fine - x0, f))],
        axis=1,
    )
    g = s_mat @ s_mat.T
    a = t_mat @ s_mat.T @ np.linalg.inv(
        g + REG * np.trace(g) / WC * np.eye(WC)
    )
    res = (xc, np.ascontiguousarray(a, dtype=np.float32))
    _interp_cache[key] = res
    return res


def _build_program(r: tuple):
    """Build the per-core Bass program (SPMD; identical on all cores).
    `r` are the 8 Fresnel coefficients, baked in as immediates."""
    if r in _program_cache:
        return _program_cache[r]

    import concourse.tile as tile
    from concourse import bacc, mybir

    f32 = mybir.dt.float32
    bf16 = mybir.dt.bfloat16
    alu = mybir.AluOpType
    act_sin = mybir.ActivationFunctionType.Sin
    act_sq = mybir.ActivationFunctionType.Square

    # Bacc (not raw Bass): its compile() splits multi-sem waits into event
    # semaphores (TRN2 allows 1 wait/instruction), moves matmul waits to
    # ldweights, inserts ACT table loads, and fills extended-ISA bytes.
    nc = bacc.Bacc()
    # k0w[h, p] = coarse k0 for half h's partitions, zero elsewhere
    k0w = nc.declare_dram_parameter("k0w", [PACK, NPART], f32, isOutput=False)
    # gp[i, h, :]: layer i, packed batch-half h
    g_p = nc.declare_dram_parameter("gp", [7, PACK, BPC // PACK], f32, isOutput=False)
    # A^T duplicated into both partition halves (rows 0:WC and WC:2*WC),
    # split hi/lo in bf16 ([0]=hi, [1]=lo) for the 3-term bf16 upsample
    amat = nc.declare_dram_parameter("Amat2", [2, NPART, W], bf16, isOutput=False)
    out = nc.declare_dram_parameter("out", [BPC, W], f32, isOutput=True)

    with tile.TileContext(nc) as tc:
        with (
            tc.tile_pool(name="const", bufs=1) as cpool,
            tc.tile_pool(name="state", bufs=2) as spool,
            tc.tile_pool(name="fin", bufs=1) as fpool,
            tc.tile_pool(name="work", bufs=2) as wpool,
            tc.tile_pool(name="thps", bufs=2, space="PSUM") as ppool,
            tc.tile_pool(name="upps", bufs=2, space="PSUM") as upool,
            tc.tile_pool(name="outp", bufs=3) as opool,
        ):
            k0_sb = cpool.tile([PACK, NPART], f32)
            nc.sync.dma_start(out=k0_sb, in_=k0w[:, :])
            a_sb = cpool.tile([NPART, 2, W], bf16)
            nc.sync.dma_start(out=a_sb, in_=amat.rearrange("s p w -> p s w"))

            for chunk in range(NCHUNK):
                ur = ui = vr = vi = None
                for i in range(7, 0, -1):
                    ri = float(r[i - 1])
                    g2 = wpool.tile([PACK, NB], f32, tag="g2")
                    nc.sync.dma_start(
                        out=g2, in_=g_p[i - 1, :, chunk * NB : (chunk + 1) * NB]
                    )
                    # theta[p, n] = k0c[p mod WC] * g_half(p)[n] (rank-2 mm;
                    # split 512-wide: a matmul may not cross a PSUM bank)
                    theta = ppool.tile([NPART, NB], f32, tag="theta")
                    for t0 in range(0, NB, 512):
                        nc.tensor.matmul(
                            theta[:, t0 : t0 + 512],
                            k0_sb,
                            g2[:, t0 : t0 + 512],
                            start=True,
                            stop=True,
                        )
                    # range-reduce into [-pi, pi]: theta in [0, ~4.2*pi)
                    psi_s = wpool.tile([NPART, NB], f32, tag="psis")
                    nc.vector.add_range_wrap(
                        psi_s, theta, shift=-TWO_PI, bound=math.pi, period=TWO_PI
                    )
                    psi_c = wpool.tile([NPART, NB], f32, tag="psic")
                    nc.vector.add_range_wrap(
                        psi_c,
                        theta,
                        shift=0.5 * math.pi - TWO_PI,
                        bound=math.pi,
                        period=TWO_PI,
                    )
                    s_t = wpool.tile([NPART, NB], f32, tag="s")
                    nc.scalar.activation(s_t, psi_s, act_sin)
                    c_t = wpool.tile([NPART, NB], f32, tag="c")
                    nc.scalar.activation(c_t, psi_c, act_sin)

                    if i == 7:
                        # closed-form first step: u=(r7*c+r8, -r7*s),
                        # v=(c+r7*r8, -s) - no memset/product chain needed
                        ur = spool.tile([NPART, NB], f32, tag="ur")
                        nc.vector.tensor_scalar(
                            ur, c_t, ri, float(r[7]), alu.mult, alu.add
                        )
                        ui = spool.tile([NPART, NB], f32, tag="ui")
                        nc.vector.tensor_scalar_mul(ui, s_t, -ri)
                        vr = spool.tile([NPART, NB], f32, tag="vr")
                        nc.vector.tensor_scalar_add(vr, c_t, ri * float(r[7]))
                        vi = spool.tile([NPART, NB], f32, tag="vi")
                        nc.vector.tensor_scalar_mul(vi, s_t, -1.0)
                        continue

                    # T = v * conj(E) : Tr = vr*c + vi*s ; Ti = vi*c - vr*s
                    p1 = wpool.tile([NPART, NB], f32, tag="p1")
                    nc.vector.tensor_mul(p1, vr, c_t)
                    p2 = wpool.tile([NPART, NB], f32, tag="p2")
                    nc.vector.tensor_mul(p2, vi, s_t)
                    # imag-T chain on the otherwise-idle GpSimd engine
                    p3 = wpool.tile([NPART, NB], f32, tag="p3")
                    nc.gpsimd.tensor_mul(p3, vi, c_t)
                    p4 = wpool.tile([NPART, NB], f32, tag="p4")
                    nc.gpsimd.tensor_mul(p4, vr, s_t)
                    t_r = wpool.tile([NPART, NB], f32, tag="Tr")
                    nc.vector.tensor_add(t_r, p1, p2)
                    t_i = wpool.tile([NPART, NB], f32, tag="Ti")
                    nc.gpsimd.tensor_sub(t_i, p3, p4)

                    # u' = r*T + u ; v' = T + r*u   (old u!)
                    ur2 = spool.tile([NPART, NB], f32, tag="ur")
                    nc.vector.scalar_tensor_tensor(ur2, t_r, ri, ur, alu.mult, alu.add)
                    ui2 = spool.tile([NPART, NB], f32, tag="ui")
                    nc.vector.scalar_tensor_tensor(ui2, t_i, ri, ui, alu.mult, alu.add)
                    vr2 = spool.tile([NPART, NB], f32, tag="vr")
                    nc.vector.scalar_tensor_tensor(vr2, ur, ri, t_r, alu.mult, alu.add)
                    vi2 = spool.tile([NPART, NB], f32, tag="vi")
                    nc.vector.scalar_tensor_tensor(vi2, ui, ri, t_i, alu.mult, alu.add)
                    ur, ui, vr, vi = ur2, ui2, vr2, vi2

                # U2 = ur^2 + ui^2 ; V2 = vr^2 + vi^2 (squares on ScalarE:
                # Square shares the trig_and_small ACT table with Sin)
                m1 = fpool.tile([NPART, NB], f32, tag="m1")
                nc.scalar.activation(m1, ur, act_sq)
                m2 = fpool.tile([NPART, NB], f32, tag="m2")
                nc.scalar.activation(m2, ui, act_sq)
                u2 = fpool.tile([NPART, NB], f32, tag="u2")
                nc.vector.tensor_add(u2, m1, m2)
                m3 = fpool.tile([NPART, NB], f32, tag="m3")
                nc.scalar.activation(m3, vr, act_sq)
                m4 = fpool.tile([NPART, NB], f32, tag="m4")
                nc.scalar.activation(m4, vi, act_sq)
                v2 = fpool.tile([NPART, NB], f32, tag="v2")
                nc.gpsimd.tensor_add(v2, m3, m4)

                # split planes into bf16 hi + lo halves: the 3-term bf16
                # matmul (hi*hi + lo*hi + hi*lo) reproduces the fp32 product
                # to ~2^-16 at ~3x less PE time than fp32's LOW_HIGH mode
                u2h = fpool.tile([NPART, NB], bf16, tag="u2h")
                nc.vector.tensor_copy(u2h, u2)
                u2l = fpool.tile([NPART, NB], bf16, tag="u2l")
                nc.vector.tensor_sub(u2l, u2, u2h)
                v2h = fpool.tile([NPART, NB], bf16, tag="v2h")
                nc.vector.tensor_copy(v2h, v2)
                v2l = fpool.tile([NPART, NB], bf16, tag="v2l")
                nc.vector.tensor_sub(v2l, v2, v2h)

                # upsample both planes per 128-row batch group; divide; store
                for half in range(PACK):
                    for bsub in range(NB // 128):
                        row0 = half * (BPC // 2) + chunk * NB + bsub * 128
                        bs = slice(bsub * 128, (bsub + 1) * 128)
                        ps = slice(half * WC, (half + 1) * WC)
                        for w0, ws in ((0, 512), (512, W - 512)):
                            a_h = a_sb[ps, 0, w0 : w0 + ws]
                            a_l = a_sb[ps, 1, w0 : w0 + ws]
                            up_u = upool.tile([128, ws], f32, tag="upu")
                            nc.tensor.matmul(
                                up_u, u2h[ps, bs], a_h, start=True, stop=False
                            )
                            nc.tensor.matmul(
                                up_u, u2h[ps, bs], a_l, start=False, stop=False
                            )
                            nc.tensor.matmul(
                                up_u, u2l[ps, bs], a_h, start=False, stop=True
                            )
                            up_v = upool.tile([128, ws], f32, tag="upv")
                            nc.tensor.matmul(
                                up_v, v2h[ps, bs], a_h, start=True, stop=False
                            )
                            nc.tensor.matmul(
                                up_v, v2h[ps, bs], a_l, start=False, stop=False
                            )
                            nc.tensor.matmul(
                                up_v, v2l[ps, bs], a_h, start=False, stop=True
                            )
                            inv = opool.tile([128, ws], f32, tag="inv")
                            nc.vector.reciprocal_approx_fast(out=inv, in_=up_v)
                            ro = opool.tile([128, ws], f32, tag="ro")
                            nc.vector.tensor_mul(ro, up_u, inv)
                            nc.sync.dma_start(
                                out=out[row0 : row0 + 128, w0 : w0 + ws], in_=ro
                            )

    # the axon/bass2jax run path asserts finalized; Bacc.finalize() also runs
    # compile() (event-sem wait splitting, reg alloc, ACT table loads, ISA bytes)
    nc.finalize()

    _program_cache[r] = nc
    return nc


def _scalar_index(arr: np.ndarray, name: str) -> float:
    """Wavelength-constant real refractive index from a [W] complex array."""
    a = np.asarray(arr)
    v = complex(a.flat[0])
    if not np.allclose(a, v, rtol=1e-6, atol=1e-6):
        raise ValueError(f"{name}: kernel requires wavelength-constant index")
    if abs(v.imag) > 1e-6 * max(1.0, abs(v.real)):
        raise ValueError(f"{name}: kernel requires a real refractive index")
    return float(v.real)


def kernel(d_phys, n_Si, n_SiO2, n_Si3N4, lam):
    from concourse.bass_utils import run_bass_kernel_spmd

    d = np.ascontiguousarray(np.asarray(d_phys), dtype=np.float32)
    assert d.shape == (B, 7), d.shape
    lam_np = np.asarray(lam, dtype=np.float64)
    assert lam_np.shape == (W,)

    nsi = _scalar_index(n_Si, "n_Si")
    nox = _scalar_index(n_SiO2, "n_SiO2")
    nni = _scalar_index(n_Si3N4, "n_Si3N4")
    layers = [1.0, nox, nni, nox, nni, nox, nni, nox, nsi]
    r = tuple(
        (layers[i - 1] - layers[i]) / (layers[i - 1] + layers[i])
        for i in range(1, 9)
    )

    # g = 2*n_i*d_i per layer, [B, 7]
    nvec = np.array(layers[1:8], dtype=np.float64)
    g = (2.0 * d.astype(np.float64) * nvec[None, :]).astype(np.float32)

    x_fine = 1.0 / lam_np
    bfit_actual = 2.0 * TWO_PI * float(g.astype(np.float64).sum(axis=1).max())
    bfit = max(B_DESIGN, bfit_actual * 1.02)
    xc, a_fine = _build_interp(x_fine, bfit)  # a_fine: [W, WC]

    # sanity: wrapped phase args must stay within +-3*pi for the one-period
    # range wrap (theta in [0, theta_max])
    theta_max = TWO_PI * float(xc.max()) * float(g.max())
    assert theta_max <= 5.0 * math.pi - 0.2, theta_max

    import ml_dtypes

    k0c = (TWO_PI * xc).astype(np.float32)
    k0w = np.zeros((PACK, NPART), dtype=np.float32)
    k0w[0, :WC] = k0c
    k0w[1, WC:] = k0c
    a2 = np.concatenate([a_fine.T, a_fine.T], axis=0)  # [2*WC, W] fp32
    a_hi = a2.astype(ml_dtypes.bfloat16)
    a_lo = (a2 - a_hi.astype(np.float32)).astype(ml_dtypes.bfloat16)
    amat2 = np.ascontiguousarray(np.stack([a_hi, a_lo], axis=0))

    g_all = g.reshape(NCORES, PACK, BPC // PACK, 7)
    in_maps = []
    for c in range(NCORES):
        # gp[i, h, :] = g of layer i for batch half h
        g_pc = np.ascontiguousarray(np.transpose(g_all[c], (2, 0, 1)))
        in_maps.append({"gp": g_pc, "k0w": k0w, "Amat2": amat2})

    nc = _build_program(r)
    _last_launch["nc"] = nc
    _last_launch["in_maps"] = in_maps
    res = run_bass_kernel_spmd(nc, in_maps, list(range(NCORES)))
    outs = [res.results[c]["out"] for c in range(NCORES)]
    return np.concatenate(outs, axis=0)


# revision 40
# speedup vs baseline: 1.2611x; 1.0283x over previous
"""Trainium2 Bass kernel for transfer-matrix reflectometry (DifferentiableKLA).

Math: for each batch row b (7 film thicknesses) and wavelength w, the
reference computes R = |M10/M00|^2 of an 8-interface transfer-matrix
product. Dividing the backward (Rouard) recurrence through by v and
rescaling by conj(E_i) each step (the overall complex scale cancels in the
ratio) gives the division-free form over x = 1/lambda:

    u, v   <- r_8, 1
    for i = 7..1:   T = v * conj(E_i);  u' = r_i*T + u;  v' = T + r_i*u
    R = |u|^2 / |v|^2,   E_i = exp(i * theta_i), theta_i = 2*pi*x*g_i,
    g_i = 2*n_i*d_i,   r_i = (n_{i-1}-n_i)/(n_{i-1}+n_i)  (real scalars
    since the refractive indices are wavelength-constant and real).

Key accel trick: U2=|u|^2 and V2=|v|^2 are *exactly band-limited* in x
(finite exponential sums, |freq| <= 4*pi*sum_j g_j), so they are computed
on a coarse WC-point grid in x and upsampled to the 801-point grid with a
precomputed band-limited least-squares matrix — evaluated as a TensorE
matmul that simultaneously transposes to the [batch, wavelength] output
layout. Only the final reciprocal+multiply runs on the fine grid.

Layout: coarse grid on partitions, batch along the free dim, two
independent batch halves packed as 2 x WC=64 partition groups (rank-2
matmul builds theta for both halves at once).
"""

import math

import numpy as np

W = 801
B = 16384
NCORES = 8
BPC = B // NCORES  # 2048 batch rows per core
WC = 64  # coarse wavelength nodes
PACK = 2  # batch halves packed along partitions
NPART = WC * PACK  # 128
NB = 512  # free-dim columns per compute chunk
NCHUNK = BPC // PACK // NB  # 2
EXT = 0.05  # coarse-grid extension fraction beyond the x band
REG = 1e-10  # LS ridge regularization
MFREQ = 6000  # frequency samples for the LS fit
TWO_PI = 2.0 * math.pi
# worst-case band limit for d < 200nm stacks (4*pi*sum(2*n_j*200)), padded
B_DESIGN = 2.0 * TWO_PI * (2.0 * (1.46 * 4 + 2.0 * 3) * 200.0) * 1.02

_interp_cache: dict = {}
_program_cache: dict = {}
_last_launch: dict = {}  # debug/profiling hook (nc, in_maps of last run)


def _build_interp(x_fine: np.ndarray, bfit: float):
    """Band-limited LS upsampling matrix A [W, WC]: rows map coarse samples
    (at xc) of any signal with spectrum in [-bfit, bfit] to the fine grid."""
    key = (x_fine.shape[0], float(x_fine[0]), float(x_fine[-1]), round(bfit))
    if key in _interp_cache:
        return _interp_cache[key]
    x0, x1 = float(x_fine.min()), float(x_fine.max())
    ln = x1 - x0
    xc = np.linspace(x0 - EXT * ln, x1 + EXT * ln, WC)
    f = np.linspace(0.0, bfit, MFREQ)
    s_mat = np.concatenate(
        [np.cos(np.outer(xc - x0, f)), np.sin(np.outer(xc - x0, f))], axis=1
    )
    t_mat = np.concatenate(
        [np.cos(np.outer(x_fine - x0, f)), np.sin(np.outer(x_fine - x0, f))],
        axis=1,
    )
    g = s_mat @ s_mat.T
    a = t_mat @ s_mat.T @ np.linalg.inv(
        g + REG * np.trace(g) / WC * np.eye(WC)
    )
    res = (xc, np.ascontiguousarray(a, dtype=np.float32))
    _interp_cache[key] = res
    return res


def _build_program(r: tuple):
    """Build the per-core Bass program (SPMD; identical on all cores).
    `r` are the 8 Fresnel coefficients, baked in as immediates."""
    if r in _program_cache:
        return _program_cache[r]

    import concourse.tile as tile
    from concourse import bacc, mybir

    f32 = mybir.dt.float32
    bf16 = mybir.dt.bfloat16
    alu = mybir.AluOpType
    act_sin = mybir.ActivationFunctionType.Sin
    act_sq = mybir.ActivationFunctionType.Square

    # Bacc (not raw Bass): its compile() splits multi-sem waits into event
    # semaphores (TRN2 allows 1 wait/instruction), moves matmul waits to
    # ldweights, inserts ACT table loads, and fills extended-ISA bytes.
    nc = bacc.Bacc()
    # k0w[h, p] = coarse k0 for half h's partitions, zero elsewhere
    k0w = nc.declare_dram_parameter("k0w", [PACK, NPART], f32, isOutput=False)
    # gp[i, h, :]: layer i, packed batch-half h
    g_p = nc.declare_dram_parameter("gp", [7, PACK, BPC // PACK], f32, isOutput=False)
    # A^T duplicated into both partition halves (rows 0:WC and WC:2*WC),
    # split hi/lo in bf16 ([0]=hi, [1]=lo) for the 3-term bf16 upsample
    amat = nc.declare_dram_parameter("Amat2", [2, NPART, W], bf16, isOutput=False)
    out = nc.declare_dram_parameter("out", [BPC, W], f32, isOutput=True)

    with tile.TileContext(nc) as tc:
        with (
            tc.tile_pool(name="const", bufs=1) as cpool,
            tc.tile_pool(name="state", bufs=2) as spool,
            tc.tile_pool(name="fin", bufs=1) as fpool,
            tc.tile_pool(name="work", bufs=2) as wpool,
            tc.tile_pool(name="thps", bufs=2, space="PSUM") as ppool,
            tc.tile_pool(name="upps", bufs=2, space="PSUM") as upool,
            tc.tile_pool(name="outp", bufs=3) as opool,
        ):
            k0_sb = cpool.tile([PACK, NPART], f32)
            nc.sync.dma_start(out=k0_sb, in_=k0w[:, :])
            g_all = cpool.tile([PACK, 7 * (BPC // PACK)], f32)
            nc.sync.dma_start(
                out=g_all.rearrange("h (i n) -> h i n", i=7),
                in_=g_p.rearrange("i h n -> h i n"),
            )
            a_sb = cpool.tile([NPART, 2, W], bf16)
            nc.sync.dma_start(out=a_sb, in_=amat.rearrange("s p w -> p s w"))

            for chunk in range(NCHUNK):
                ur = ui = vr = vi = None
                for i in range(7, 0, -1):
                    ri = float(r[i - 1])
                    # theta[p, n] = k0c[p mod WC] * g_half(p)[n] (rank-2 mm;
                    # split 512-wide: a matmul may not cross a PSUM bank)
                    theta = ppool.tile([NPART, NB], f32, tag="theta")
                    gbase = (i - 1) * (BPC // PACK) + chunk * NB
                    for t0 in range(0, NB, 512):
                        nc.tensor.matmul(
                            theta[:, t0 : t0 + 512],
                            k0_sb,
                            g_all[:, gbase + t0 : gbase + t0 + 512],
                            start=True,
                            stop=True,
                        )
                    # range-reduce into [-pi, pi]: theta in [0, ~4.2*pi)
                    psi_s = wpool.tile([NPART, NB], f32, tag="psis")
                    nc.vector.add_range_wrap(
                        psi_s, theta, shift=-TWO_PI, bound=math.pi, period=TWO_PI
                    )
                    psi_c = wpool.tile([NPART, NB], f32, tag="psic")
                    nc.vector.add_range_wrap(
                        psi_c,
                        theta,
                        shift=0.5 * math.pi - TWO_PI,
                        bound=math.pi,
                        period=TWO_PI,
                    )
                    s_t = wpool.tile([NPART, NB], f32, tag="s")
                    nc.scalar.activation(s_t, psi_s, act_sin)
                    c_t = wpool.tile([NPART, NB], f32, tag="c")
                    nc.scalar.activation(c_t, psi_c, act_sin)

                    if i == 7:
                        # closed-form first step: u=(r7*c+r8, -r7*s),
                        # v=(c+r7*r8, -s) - no memset/product chain needed
                        ur = spool.tile([NPART, NB], f32, tag="ur")
                        nc.vector.tensor_scalar(
                            ur, c_t, ri, float(r[7]), alu.mult, alu.add
                        )
                        ui = spool.tile([NPART, NB], f32, tag="ui")
                        nc.vector.tensor_scalar_mul(ui, s_t, -ri)
                        vr = spool.tile([NPART, NB], f32, tag="vr")
                        nc.vector.tensor_scalar_add(vr, c_t, ri * float(r[7]))
                        vi = spool.tile([NPART, NB], f32, tag="vi")
                        nc.vector.tensor_scalar_mul(vi, s_t, -1.0)
                        continue

                    # T = v * conj(E) : Tr = vr*c + vi*s ; Ti = vi*c - vr*s
                    p1 = wpool.tile([NPART, NB], f32, tag="p1")
                    nc.vector.tensor_mul(p1, vr, c_t)
                    p2 = wpool.tile([NPART, NB], f32, tag="p2")
                    nc.vector.tensor_mul(p2, vi, s_t)
                    # imag-T chain on the otherwise-idle GpSimd engine
                    p3 = wpool.tile([NPART, NB], f32, tag="p3")
                    nc.gpsimd.tensor_mul(p3, vi, c_t)
                    p4 = wpool.tile([NPART, NB], f32, tag="p4")
                    nc.gpsimd.tensor_mul(p4, vr, s_t)
                    t_r = wpool.tile([NPART, NB], f32, tag="Tr")
                    nc.vector.tensor_add(t_r, p1, p2)
                    t_i = wpool.tile([NPART, NB], f32, tag="Ti")
                    nc.vector.tensor_sub(t_i, p3, p4)

                    # u' = r*T + u ; v' = T + r*u   (old u!)
                    ur2 = spool.tile([NPART, NB], f32, tag="ur")
                    nc.vector.scalar_tensor_tensor(ur2, t_r, ri, ur, alu.mult, alu.add)
                    ui2 = spool.tile([NPART, NB], f32, tag="ui")
                    nc.vector.scalar_tensor_tensor(ui2, t_i, ri, ui, alu.mult, alu.add)
                    vr2 = spool.tile([NPART, NB], f32, tag="vr")
                    nc.vector.scalar_tensor_tensor(vr2, ur, ri, t_r, alu.mult, alu.add)
                    vi2 = spool.tile([NPART, NB], f32, tag="vi")
                    nc.vector.scalar_tensor_tensor(vi2, ui, ri, t_i, alu.mult, alu.add)
                    ur, ui, vr, vi = ur2, ui2, vr2, vi2

                # U2 = ur^2 + ui^2 ; V2 = vr^2 + vi^2 (squares on ScalarE:
                # Square shares the trig_and_small ACT table with Sin)
                m1 = fpool.tile([NPART, NB], f32, tag="m1")
                nc.scalar.activation(m1, ur, act_sq)
                m2 = fpool.tile([NPART, NB], f32, tag="m2")
                nc.scalar.activation(m2, ui, act_sq)
                u2 = fpool.tile([NPART, NB], f32, tag="u2")
                nc.vector.tensor_add(u2, m1, m2)
                m3 = fpool.tile([NPART, NB], f32, tag="m3")
                nc.scalar.activation(m3, vr, act_sq)
                m4 = fpool.tile([NPART, NB], f32, tag="m4")
                nc.scalar.activation(m4, vi, act_sq)
                v2 = fpool.tile([NPART, NB], f32, tag="v2")
                nc.gpsimd.tensor_add(v2, m3, m4)

                # split planes into bf16 hi + lo halves: the 3-term bf16
                # matmul (hi*hi + lo*hi + hi*lo) reproduces the fp32 product
                # to ~2^-16 at ~3x less PE time than fp32's LOW_HIGH mode
                u2h = fpool.tile([NPART, NB], bf16, tag="u2h")
                nc.vector.tensor_copy(u2h, u2)
                u2l = fpool.tile([NPART, NB], bf16, tag="u2l")
                nc.vector.tensor_sub(u2l, u2, u2h)
                v2h = fpool.tile([NPART, NB], bf16, tag="v2h")
                nc.vector.tensor_copy(v2h, v2)
                v2l = fpool.tile([NPART, NB], bf16, tag="v2l")
                nc.vector.tensor_sub(v2l, v2, v2h)

                # upsample both planes per 128-row batch group; divide; store
                for half in range(PACK):
                    for bsub in range(NB // 128):
                        row0 = half * (BPC // 2) + chunk * NB + bsub * 128
                        bs = slice(bsub * 128, (bsub + 1) * 128)
                        ps = slice(half * WC, (half + 1) * WC)
                        for w0, ws in ((0, 512), (512, W - 512)):
                            a_h = a_sb[ps, 0, w0 : w0 + ws]
                            a_l = a_sb[ps, 1, w0 : w0 + ws]
                            up_u = upool.tile([128, ws], f32, tag="upu")
                            nc.tensor.matmul(
                                up_u, u2h[ps, bs], a_h, start=True, stop=False
                            )
                            nc.tensor.matmul(
                                up_u, u2h[ps, bs], a_l, start=False, stop=False
                            )
                            nc.tensor.matmul(
                                up_u, u2l[ps, bs], a_h, start=False, stop=True
                            )
                            up_v = upool.tile([128, ws], f32, tag="upv")
                            nc.tensor.matmul(
                                up_v, v2h[ps, bs], a_h, start=True, stop=False
                            )
                            nc.tensor.matmul(
                                up_v, v2h[ps, bs], a_l, start=False, stop=False
                            )
                            nc.tensor.matmul(
                                up_v, v2l[ps, bs], a_h, start=False, stop=True
                            )
                            inv = opool.tile([128, ws], f32, tag="inv")
                            nc.vector.reciprocal_approx_fast(out=inv, in_=up_v)
                            ro = opool.tile([128, ws], f32, tag="ro")
                            nc.vector.tensor_mul(ro, up_u, inv)
                            nc.sync.dma_start(
                                out=out[row0 : row0 + 128, w0 : w0 + ws], in_=ro
                            )

    # the axon/bass2jax run path asserts finalized; Bacc.finalize() also runs
    # compile() (event-sem wait splitting, reg alloc, ACT table loads, ISA bytes)
    nc.finalize()

    _program_cache[r] = nc
    return nc


def _scalar_index(arr: np.ndarray, name: str) -> float:
    """Wavelength-constant real refractive index from a [W] complex array."""
    a = np.asarray(arr)
    v = complex(a.flat[0])
    if not np.allclose(a, v, rtol=1e-6, atol=1e-6):
        raise ValueError(f"{name}: kernel requires wavelength-constant index")
    if abs(v.imag) > 1e-6 * max(1.0, abs(v.real)):
        raise ValueError(f"{name}: kernel requires a real refractive index")
    return float(v.real)


def kernel(d_phys, n_Si, n_SiO2, n_Si3N4, lam):
    from concourse.bass_utils import run_bass_kernel_spmd

    d = np.ascontiguousarray(np.asarray(d_phys), dtype=np.float32)
    assert d.shape == (B, 7), d.shape
    lam_np = np.asarray(lam, dtype=np.float64)
    assert lam_np.shape == (W,)

    nsi = _scalar_index(n_Si, "n_Si")
    nox = _scalar_index(n_SiO2, "n_SiO2")
    nni = _scalar_index(n_Si3N4, "n_Si3N4")
    layers = [1.0, nox, nni, nox, nni, nox, nni, nox, nsi]
    r = tuple(
        (layers[i - 1] - layers[i]) / (layers[i - 1] + layers[i])
        for i in range(1, 9)
    )

    # g = 2*n_i*d_i per layer, [B, 7]
    nvec = np.array(layers[1:8], dtype=np.float64)
    g = (2.0 * d.astype(np.float64) * nvec[None, :]).astype(np.float32)

    x_fine = 1.0 / lam_np
    bfit_actual = 2.0 * TWO_PI * float(g.astype(np.float64).sum(axis=1).max())
    bfit = max(B_DESIGN, bfit_actual * 1.02)
    xc, a_fine = _build_interp(x_fine, bfit)  # a_fine: [W, WC]

    # sanity: wrapped phase args must stay within +-3*pi for the one-period
    # range wrap (theta in [0, theta_max])
    theta_max = TWO_PI * float(xc.max()) * float(g.max())
    assert theta_max <= 5.0 * math.pi - 0.2, theta_max

    import ml_dtypes

    k0c = (TWO_PI * xc).astype(np.float32)
    k0w = np.zeros((PACK, NPART), dtype=np.float32)
    k0w[0, :WC] = k0c
    k0w[1, WC:] = k0c
    a2 = np.concatenate([a_fine.T, a_fine.T], axis=0)  # [2*WC, W] fp32
    a_hi = a2.astype(ml_dtypes.bfloat16)
    a_lo = (a2 - a_hi.astype(np.float32)).astype(ml_dtypes.bfloat16)
    amat2 = np.ascontiguousarray(np.stack([a_hi, a_lo], axis=0))

    g_all = g.reshape(NCORES, PACK, BPC // PACK, 7)
    in_maps = []
    for c in range(NCORES):
        # gp[i, h, :] = g of layer i for batch half h
        g_pc = np.ascontiguousarray(np.transpose(g_all[c], (2, 0, 1)))
        in_maps.append({"gp": g_pc, "k0w": k0w, "Amat2": amat2})

    nc = _build_program(r)
    _last_launch["nc"] = nc
    _last_launch["in_maps"] = in_maps
    res = run_bass_kernel_spmd(nc, in_maps, list(range(NCORES)))
    outs = [res.results[c]["out"] for c in range(NCORES)]
    return np.concatenate(outs, axis=0)
